# revision 4
# baseline (speedup 1.0000x reference)
"""Multi-head attention (B=4, S=2048, D=1024, H=16) on 8 trn2 NeuronCores.

Sharding: core c = (batch b, head-group g) with b in 0..3, g in 0..1.
Each core computes 8 heads of one batch; the two cores of a batch produce
partial output projections that the host sums.

All device tensors are kept in "transposed" layouts (feature dim on SBUF
partitions) so no on-device transposes are needed:
  Q^T/K^T [d, s], V [s, d] (+ones col), scores^T [k, q], o^T [d, q], y^T [out, q].
Softmax uses no max-subtraction (scores bounded ~ +-5 for this regime) and
the denominator comes from an appended ones-column in the PV matmul.
"""
import math

import numpy as np
import ml_dtypes

import concourse.bass as bass
import concourse.mybir as mybir
import concourse.tile as tile
from concourse import bacc
from concourse.bass_utils import run_bass_kernel_spmd

B, S, D, H = 4, 2048, 1024, 16
DK = D // H              # 64
NCORES = 8
HG = 2                   # head groups (tensor-parallel axis)
HPG = H // HG            # 8 heads per core
HD = HPG * DK            # 512 head-dim features per core
PAIRS = HPG // 2         # 4 head pairs (2 heads row-packed per PE pass)
P = 128
QC = 512                 # q-chunk (matmul moving free dim)
NQC = S // QC            # 4
NKT = S // P             # 16 k-tiles
FK = D // P              # 8 feature c-tiles for projections
TC = 512                 # token chunk for QKV phase
NTC = S // TC            # 4

F32 = mybir.dt.float32
F32R = mybir.dt.float32r
BF16 = mybir.dt.bfloat16

LAST_EXEC_NS = None


def _build(apply_mask: bool, qkv_bias: bool):
    nc = bacc.Bacc("TRN2", debug=False, num_devices=NCORES)
    xT = nc.declare_dram_parameter("xT", [D, S], F32R, isOutput=False)
    wqkv = nc.declare_dram_parameter("wqkv", [D, 3 * HD], F32R, isOutput=False)
    wo = nc.declare_dram_parameter("wo", [HD, D], BF16, isOutput=False)
    yT = nc.declare_dram_parameter("yT", [D, S], F32, isOutput=True)
    if apply_mask:
        maskT = nc.declare_dram_parameter("maskT", [S, S], F32, isOutput=False)
    if qkv_bias:
        qkb = nc.declare_dram_parameter("qkb", [2, HD], F32, isOutput=False)
        vb = nc.declare_dram_parameter("vb", [HD], F32, isOutput=False)

    xT_r = xT.rearrange("(fo p) s -> p fo s", p=P)       # [128, 8, 2048]
    wqkv_r = wqkv.rearrange("(fo p) n -> p fo n", p=P)   # [128, 8, 1536]
    wo_r = wo.rearrange("(co p) n -> p co n", p=P)       # [128, 4, 1024]
    yT_r = yT.rearrange("(oo p) s -> p oo s", p=P)       # [128, 8, 2048]

    phat_bufs = 1 if apply_mask else 2

    with tile.TileContext(nc) as tc:
        with tc.tile_pool(name="persist", bufs=1) as persist, \
             tc.tile_pool(name="work", bufs=2) as work, \
             tc.tile_pool(name="small", bufs=3) as small, \
             tc.tile_pool(name="ps", bufs=2, space="PSUM") as ps:

            QT = persist.tile([P, PAIRS, S], F32R)        # 32KB/part
            KTt = persist.tile([P, PAIRS, S], F32R)       # 32KB/part
            V = persist.tile([P, NKT, HPG * (DK + 1)], BF16)  # 16.25KB/part
            wo_t = persist.tile([P, HD // P, D], BF16)    # 8KB/part
            nc.sync.dma_start(wo_t, wo_r)

            if qkv_bias:
                qkb_t = persist.tile([P, 2, PAIRS], F32)   # [d-part, {q,k}, pair]
                nc.sync.dma_start(
                    qkb_t, qkb.rearrange("t (pr p) -> p t pr", p=P))
                vb_bc = persist.tile([P, HD], F32)
                nc.sync.dma_start(vb_bc, vb[None, :].partition_broadcast(P))

            # ones columns of V (softmax-denominator trick)
            for h in range(HPG):
                nc.vector.memset(V[:, :, h * (DK + 1) + DK], 1.0)

            # ---------------- Phase 1: QKV projections -----------------
            with tc.tile_pool(name="wpool", bufs=1) as wpool, \
                 tc.tile_pool(name="xpool", bufs=2) as xpool:
                wqkv_t = wpool.tile([P, FK, 3 * HD], F32R)  # 48KB/part
                nc.sync.dma_start(wqkv_t, wqkv_r)

                for tcix in range(NTC):
                    tsl = slice(tcix * TC, (tcix + 1) * TC)
                    xt = xpool.tile([P, FK, TC], F32R, tag="xt")  # 16KB/part
                    nc.sync.dma_start(xt, xT_r[:, :, tsl])

                    # Q^T and K^T: [d-pair 128, tokens 512] accumulating over f
                    for which, base in ((0, 0), (1, HD)):
                        for pair in range(PAIRS):
                            psqk = ps.tile([P, TC], F32, tag="qkv")
                            msl = slice(base + pair * P, base + (pair + 1) * P)
                            for ko in range(FK):
                                nc.tensor.matmul(
                                    psqk, wqkv_t[:, ko, msl], xt[:, ko],
                                    start=(ko == 0), stop=(ko == FK - 1))
                            dst = (QT if which == 0 else KTt)[:, pair, tsl]
                            if qkv_bias:
                                nc.vector.tensor_scalar_add(
                                    dst, psqk, qkb_t[:, which, pair, None])
                            else:
                                nc.vector.tensor_copy(dst, psqk)

                    # V: [tokens 128, d 512] accumulating over f
                    for tt in range(TC // P):
                        kt = tcix * (TC // P) + tt
                        psv = ps.tile([P, HD], F32, tag="qkv")
                        for ko in range(FK):
                            nc.tensor.matmul(
                                psv, xt[:, ko, tt * P:(tt + 1) * P],
                                wqkv_t[:, ko, 2 * HD:3 * HD],
                                start=(ko == 0), stop=(ko == FK - 1))
                        # scatter into per-head 65-wide planes (col 64 = ones)
                        vdst = V[:, kt, :].rearrange(
                            "p (h w) -> p h w", h=HPG)[:, :, :DK]
                        vsrc = psv.rearrange("p (h w) -> p h w", h=HPG)
                        if qkv_bias:
                            nc.vector.tensor_add(
                                vdst, vsrc,
                                vb_bc.rearrange("p (h w) -> p h w", h=HPG))
                        else:
                            nc.vector.tensor_copy(vdst, vsrc)

            # ---------------- Phase 2: attention + out-projection -------
            with tc.tile_pool(name="phat", bufs=phat_bufs) as phatp, \
                 tc.tile_pool(name="opool", bufs=2) as opool:
                for qc in range(NQC):
                    qsl = slice(qc * QC, (qc + 1) * QC)
                    o_bf = opool.tile([P, HD // P, QC], BF16, tag="o_sb")
                    if apply_mask:
                        mt = opool.tile([P, NKT, QC], F32, tag="mask")
                        nc.sync.dma_start(
                            mt,
                            maskT.rearrange("(ko p) q -> p ko q", p=P)[:, :, qsl])
                    for pair in range(PAIRS):
                        phat = phatp.tile([P, NKT, 2 * QC], BF16)  # 32KB/part
                        for kt in range(NKT):
                            ksl = slice(kt * P, (kt + 1) * P)
                            pss = ps.tile([P, 2 * QC], F32, tag="scores")
                            # head A (even): d-rows 0..63; head B: 64..127
                            nc.tensor.matmul(
                                pss[:, 0:QC], KTt[0:DK, pair, ksl],
                                QT[0:DK, pair, qsl], start=True, stop=True)
                            nc.tensor.matmul(
                                pss[:, QC:2 * QC], KTt[DK:P, pair, ksl],
                                QT[DK:P, pair, qsl], start=True, stop=True)
                            if apply_mask:
                                nc.vector.tensor_add(
                                    pss[:, 0:QC], pss[:, 0:QC], mt[:, kt])
                                nc.vector.tensor_add(
                                    pss[:, QC:2 * QC], pss[:, QC:2 * QC],
                                    mt[:, kt])
                            nc.scalar.activation(
                                phat[:, kt, :], pss,
                                mybir.ActivationFunctionType.Exp)

                        for half in range(2):
                            hh = 2 * pair + half  # head index within group
                            vcol = slice(hh * (DK + 1), (hh + 1) * (DK + 1))
                            pso = ps.tile([P, QC], F32, tag="o")
                            for kt in range(NKT):
                                nc.tensor.matmul(
                                    pso[0:DK + 1, :], V[:, kt, vcol],
                                    phat[:, kt, half * QC:(half + 1) * QC],
                                    start=(kt == 0), stop=(kt == NKT - 1))
                            # normalize: o[d, q] * (1 / l[q])
                            r_sb = small.tile([1, QC], F32, tag="r")
                            nc.vector.reciprocal(r_sb, pso[DK:DK + 1, :])
                            r_bc = small.tile([DK, QC], F32, tag="rbc")
                            nc.gpsimd.partition_broadcast(r_bc, r_sb)
                            nc.vector.tensor_mul(
                                o_bf[half * DK:(half + 1) * DK, pair, :],
                                pso[0:DK, :], r_bc)

                    # output projection for this q-chunk
                    for oc in range(D // P):
                        psy = ps.tile([P, QC], F32, tag="qkv")
                        for c in range(HD // P):
                            nc.tensor.matmul(
                                psy, wo_t[:, c, oc * P:(oc + 1) * P],
                                o_bf[:, c, :],
                                start=(c == 0), stop=(c == HD // P - 1))
                        yst = work.tile([P, QC], F32, tag="y")
                        nc.vector.tensor_copy(yst, psy)
                        nc.sync.dma_start(yT_r[:, oc, qsl], yst)

    nc.finalize()
    return nc


# --------------------------------------------------------------------------
# NTFF profiling shim (only used when kernel(..., _trace=True); provides
# antenv.axon_hooks so run_bass_kernel_spmd can capture profiles under axon).
def _install_ntff_shim():
    import contextlib, ctypes, sys, types
    try:
        import antenv.axon_hooks  # noqa: F401
        return
    except ImportError:
        pass
    so = "/opt/axon/libaxon_pjrt.so"
    try:
        lib = ctypes.CDLL(so)
    except OSError:
        return
    if not hasattr(lib, "axon_start_nrt_profile"):
        return
    lib.axon_start_nrt_profile.argtypes = [
        ctypes.POINTER(ctypes.c_int64), ctypes.c_size_t]
    lib.axon_start_nrt_profile.restype = ctypes.c_int64
    lib.axon_stop_nrt_profile.argtypes = [ctypes.c_char_p]
    lib.axon_stop_nrt_profile.restype = ctypes.c_int64

    @contextlib.contextmanager
    def _hook(output_dir, device_ids):
        import jax
        jax.devices()
        if device_ids:
            ids = (ctypes.c_int64 * len(device_ids))(*device_ids)
            rc = lib.axon_start_nrt_profile(ids, len(device_ids))
        else:
            rc = lib.axon_start_nrt_profile(None, 0)
        if rc != 0:
            raise RuntimeError(f"axon_start_nrt_profile rc={rc}")
        try:
            yield
        finally:
            n = lib.axon_stop_nrt_profile(str(output_dir).encode())
            print(f"ntff: {n} profile file(s) in {output_dir}", file=sys.stderr)

    import antenv
    mod = types.ModuleType("antenv.axon_hooks")
    mod.get_axon_ntff_profile_hook = lambda: _hook
    mod.set_axon_ntff_profile_hook = lambda h: None
    sys.modules["antenv.axon_hooks"] = mod
    antenv.axon_hooks = mod


def kernel(x, mask, Wq, bq, Wk, bk, Wv, bv, Wo, bo, _trace=False):
    global LAST_EXEC_NS
    x = np.ascontiguousarray(np.asarray(x, dtype=np.float32))
    mask = np.asarray(mask)
    Wq = np.asarray(Wq, dtype=np.float32)
    Wk = np.asarray(Wk, dtype=np.float32)
    Wv = np.asarray(Wv, dtype=np.float32)
    Wo = np.asarray(Wo, dtype=np.float32)
    bq = np.asarray(bq, dtype=np.float32)
    bk = np.asarray(bk, dtype=np.float32)
    bv = np.asarray(bv, dtype=np.float32)
    bo = np.asarray(bo, dtype=np.float32)

    scale = np.float32(1.0 / math.sqrt(DK))
    apply_mask = not bool((mask != 0).all())
    qkv_bias = bool(bq.any() or bk.any() or bv.any())

    nc = _build(apply_mask, qkv_bias)

    if apply_mask:
        mbias = np.where(mask == 0, np.float32(-1e9), np.float32(0.0))
        # maskT[b][k, q] = mbias[b][q, k]
        maskT = np.ascontiguousarray(np.transpose(mbias, (0, 2, 1)))

    in_maps = []
    for b in range(B):
        xT_np = np.ascontiguousarray(x[b].T)  # [D, S]
        for g in range(HG):
            rows = slice(g * HD, (g + 1) * HD)
            wqkv_np = np.ascontiguousarray(np.concatenate(
                [Wq[rows].T * scale, Wk[rows].T, Wv[rows].T], axis=1))
            wo_np = np.ascontiguousarray(
                Wo[:, rows].T).astype(ml_dtypes.bfloat16)
            m = {"xT": xT_np, "wqkv": wqkv_np, "wo": wo_np}
            if apply_mask:
                m["maskT"] = maskT[b]
            if qkv_bias:
                m["qkb"] = np.ascontiguousarray(
                    np.stack([bq[rows] * scale, bk[rows]]))
                m["vb"] = np.ascontiguousarray(bv[rows])
            in_maps.append(m)

    if _trace:
        _install_ntff_shim()
    r = run_bass_kernel_spmd(nc, in_maps, list(range(NCORES)), trace=_trace)
    LAST_EXEC_NS = r.exec_time_ns

    y = np.empty((B, S, D), dtype=np.float32)
    for b in range(B):
        yT = r.results[2 * b]["yT"] + r.results[2 * b + 1]["yT"]
        y[b] = yT.T + bo[None, :]
    return y


# revision 9
# speedup vs baseline: 1.1020x; 1.1020x over previous
"""Multi-head attention (B=4, S=2048, D=1024, H=16) on 8 trn2 NeuronCores.

Sharding: core c = (batch b, head-group g) with b in 0..3, g in 0..1.
Each core computes 8 heads of one batch; the two cores of a batch produce
partial output projections that the host sums.

All device tensors are kept in "transposed" layouts (feature dim on SBUF
partitions) so no on-device transposes are needed:
  Q^T/K^T [d, s], V [s, d] (+ones col), scores^T [k, q], o^T [d, q], y^T [out, q].
Softmax uses no max-subtraction (scores bounded ~ +-5 for this regime) and
the denominator comes from an appended ones-column in the PV matmul.
"""
import math

import numpy as np
import ml_dtypes

import concourse.bass as bass
import concourse.mybir as mybir
import concourse.tile as tile
from concourse import bacc
from concourse.bass_utils import run_bass_kernel_spmd

B, S, D, H = 4, 2048, 1024, 16
DK = D // H              # 64
NCORES = 8
HG = 2                   # head groups (tensor-parallel axis)
HPG = H // HG            # 8 heads per core
HD = HPG * DK            # 512 head-dim features per core
PAIRS = HPG // 2         # 4 head pairs (2 heads row-packed per PE pass)
P = 128
QC = 512                 # q-chunk (matmul moving free dim)
NQC = S // QC            # 4
NKT = S // P             # 16 k-tiles
FK = D // P              # 8 feature c-tiles for projections
TC = 512                 # token chunk for QKV phase
NTC = S // TC            # 4

F32 = mybir.dt.float32
F32R = mybir.dt.float32r
BF16 = mybir.dt.bfloat16

LAST_EXEC_NS = None


def _build(apply_mask: bool, qkv_bias: bool):
    nc = bacc.Bacc("TRN2", debug=False, num_devices=NCORES)
    xT = nc.declare_dram_parameter("xT", [D, S], F32R, isOutput=False)
    wqkv = nc.declare_dram_parameter("wqkv", [D, 3 * HD], F32R, isOutput=False)
    wo = nc.declare_dram_parameter("wo", [HD, D], BF16, isOutput=False)
    yT = nc.declare_dram_parameter("yT", [D, S], F32, isOutput=True)
    if apply_mask:
        maskT = nc.declare_dram_parameter("maskT", [S, S], F32, isOutput=False)
    if qkv_bias:
        qkb = nc.declare_dram_parameter("qkb", [2, HD], F32, isOutput=False)
        vb = nc.declare_dram_parameter("vb", [HD], F32, isOutput=False)

    xT_r = xT.rearrange("(fo p) s -> p fo s", p=P)       # [128, 8, 2048]
    wqkv_r = wqkv.rearrange("(fo p) n -> p fo n", p=P)   # [128, 8, 1536]
    wo_r = wo.rearrange("(co p) n -> p co n", p=P)       # [128, 4, 1024]
    yT_r = yT.rearrange("(oo p) s -> p oo s", p=P)       # [128, 8, 2048]

    phat_bufs = 1 if apply_mask else 2

    with tile.TileContext(nc) as tc:
        with tc.tile_pool(name="persist", bufs=1) as persist, \
             tc.tile_pool(name="work", bufs=2) as work, \
             tc.tile_pool(name="small", bufs=3) as small, \
             tc.tile_pool(name="ps", bufs=2, space="PSUM") as ps:

            QT = persist.tile([P, PAIRS, S], F32R)        # 32KB/part
            KTt = persist.tile([P, PAIRS, S], F32R)       # 32KB/part
            V = persist.tile([P, NKT, HPG * (DK + 1)], BF16)  # 16.25KB/part
            wo_t = persist.tile([P, HD // P, D], BF16)    # 8KB/part
            nc.sync.dma_start(wo_t, wo_r)

            if qkv_bias:
                qkb_t = persist.tile([P, 2, PAIRS], F32)   # [d-part, {q,k}, pair]
                nc.sync.dma_start(
                    qkb_t, qkb.rearrange("t (pr p) -> p t pr", p=P))
                vb_bc = persist.tile([P, HD], F32)
                nc.sync.dma_start(vb_bc, vb[None, :].partition_broadcast(P))

            # ones columns of V (softmax-denominator trick)
            for h in range(HPG):
                nc.vector.memset(V[:, :, h * (DK + 1) + DK], 1.0)

            # ---------------- Phase 1: QKV projections -----------------
            with tc.tile_pool(name="wpool", bufs=1) as wpool, \
                 tc.tile_pool(name="xpool", bufs=2) as xpool:
                wqkv_t = wpool.tile([P, FK, 3 * HD], F32R)  # 48KB/part
                nc.sync.dma_start(wqkv_t, wqkv_r)

                for tcix in range(NTC):
                    tsl = slice(tcix * TC, (tcix + 1) * TC)
                    xt = xpool.tile([P, FK, TC], F32R, tag="xt")  # 16KB/part
                    nc.sync.dma_start(xt, xT_r[:, :, tsl])

                    # Q^T and K^T: [d-pair 128, tokens 512] accumulating over f
                    for which, base in ((0, 0), (1, HD)):
                        for pair in range(PAIRS):
                            psqk = ps.tile([P, 2 * QC], F32, tag="o", name="psqk")[:, :TC]
                            msl = slice(base + pair * P, base + (pair + 1) * P)
                            for ko in range(FK):
                                nc.tensor.matmul(
                                    psqk, wqkv_t[:, ko, msl], xt[:, ko],
                                    start=(ko == 0), stop=(ko == FK - 1))
                            dst = (QT if which == 0 else KTt)[:, pair, tsl]
                            if qkv_bias:
                                nc.vector.tensor_scalar_add(
                                    dst, psqk, qkb_t[:, which, pair, None])
                            else:
                                nc.vector.tensor_copy(dst, psqk)

                    # V: [tokens 128, d 512] accumulating over f
                    for tt in range(TC // P):
                        kt = tcix * (TC // P) + tt
                        psv = ps.tile([P, 2 * QC], F32, tag="o", name="psv")[:, :HD]
                        for ko in range(FK):
                            nc.tensor.matmul(
                                psv, xt[:, ko, tt * P:(tt + 1) * P],
                                wqkv_t[:, ko, 2 * HD:3 * HD],
                                start=(ko == 0), stop=(ko == FK - 1))
                        # scatter into per-head 65-wide planes (col 64 = ones)
                        vdst = V[:, kt, :].rearrange(
                            "p (h w) -> p h w", h=HPG)[:, :, :DK]
                        vsrc = psv.rearrange("p (h w) -> p h w", h=HPG)
                        if qkv_bias:
                            nc.vector.tensor_add(
                                vdst, vsrc,
                                vb_bc.rearrange("p (h w) -> p h w", h=HPG))
                        else:
                            nc.vector.tensor_copy(vdst, vsrc)

            # ---------------- Phase 2: attention + out-projection -------
            # Software pipeline: while pair i's scores stream through
            # PE->ACT(exp), pair i-1's PV matmuls fill PE's slack so the PE
            # never idles long enough to re-throttle (HAM).
            with tc.tile_pool(name="phat", bufs=phat_bufs) as phatp, \
                 tc.tile_pool(name="opool", bufs=2) as opool:

                o_tiles = {}

                def emit_tail(st, pso):
                    """Normalize prev pair's o by 1/l and write bf16 o^T."""
                    for half in range(2):
                        hsl = slice(half * QC, (half + 1) * QC)
                        r_sb = small.tile([1, QC], F32, tag="r")
                        nc.vector.reciprocal(r_sb, pso[DK:DK + 1, hsl])
                        r_bc = small.tile([DK, QC], F32, tag="rbc")
                        nc.gpsimd.partition_broadcast(r_bc, r_sb)
                        nc.vector.tensor_mul(
                            o_tiles[st["qc"]][half * DK:(half + 1) * DK,
                                              st["pair"], :],
                            pso[0:DK, hsl], r_bc)

                def emit_proj(qc):
                    qsl = slice(qc * QC, (qc + 1) * QC)
                    o_bf = o_tiles[qc]
                    for oc in range(D // P):
                        psy = ps.tile([P, 2 * QC], F32, tag="o", name="psy")[:, :QC]
                        for c in range(HD // P):
                            nc.tensor.matmul(
                                psy, wo_t[:, c, oc * P:(oc + 1) * P],
                                o_bf[:, c, :],
                                start=(c == 0), stop=(c == HD // P - 1))
                        yst = work.tile([P, QC], F32, tag="y")
                        nc.vector.tensor_copy(yst, psy)
                        nc.sync.dma_start(yT_r[:, oc, qsl], yst)

                prev = None
                for qc in range(NQC):
                    qsl = slice(qc * QC, (qc + 1) * QC)
                    o_tiles[qc] = opool.tile(
                        [P, HD // P, QC], BF16, tag="o_sb", name="o_sb")
                    if apply_mask:
                        mt = opool.tile([P, NKT, QC], F32, tag="mask")
                        nc.sync.dma_start(
                            mt,
                            maskT.rearrange("(ko p) q -> p ko q", p=P)[:, :, qsl])
                    for pair in range(PAIRS):
                        phat = phatp.tile([P, NKT, 2 * QC], BF16)  # 32KB/part
                        pso = (ps.tile([P, 2 * QC], F32, tag="o", name="pso")
                               if prev is not None else None)
                        for kt in range(NKT):
                            ksl = slice(kt * P, (kt + 1) * P)
                            pss = ps.tile([P, 2 * QC], F32, tag="scores")
                            # head A (even): d-rows 0..63; head B: 64..127
                            nc.tensor.matmul(
                                pss[:, 0:QC], KTt[0:DK, pair, ksl],
                                QT[0:DK, pair, qsl], start=True, stop=True)
                            nc.tensor.matmul(
                                pss[:, QC:2 * QC], KTt[DK:P, pair, ksl],
                                QT[DK:P, pair, qsl], start=True, stop=True)
                            if apply_mask:
                                nc.vector.tensor_add(
                                    pss[:, 0:QC], pss[:, 0:QC], mt[:, kt])
                                nc.vector.tensor_add(
                                    pss[:, QC:2 * QC], pss[:, QC:2 * QC],
                                    mt[:, kt])
                            nc.scalar.activation(
                                phat[:, kt, :], pss,
                                mybir.ActivationFunctionType.Exp)
                            if prev is not None:
                                for half in range(2):
                                    hh = 2 * prev["pair"] + half
                                    vcol = slice(hh * (DK + 1),
                                                 (hh + 1) * (DK + 1))
                                    nc.tensor.matmul(
                                        pso[0:DK + 1,
                                            half * QC:(half + 1) * QC],
                                        V[:, kt, vcol],
                                        prev["phat"][:, kt,
                                                     half * QC:(half + 1) * QC],
                                        start=(kt == 0), stop=(kt == NKT - 1))
                        if prev is not None:
                            emit_tail(prev, pso)
                            if prev["pair"] == PAIRS - 1:
                                emit_proj(prev["qc"])
                        prev = {"qc": qc, "pair": pair, "phat": phat}

                # epilogue: PV + tail for the last pair, proj for last qc
                pso = ps.tile([P, 2 * QC], F32, tag="o")
                for kt in range(NKT):
                    for half in range(2):
                        hh = 2 * prev["pair"] + half
                        vcol = slice(hh * (DK + 1), (hh + 1) * (DK + 1))
                        nc.tensor.matmul(
                            pso[0:DK + 1, half * QC:(half + 1) * QC],
                            V[:, kt, vcol],
                            prev["phat"][:, kt, half * QC:(half + 1) * QC],
                            start=(kt == 0), stop=(kt == NKT - 1))
                emit_tail(prev, pso)
                emit_proj(prev["qc"])

    nc.finalize()
    return nc


# --------------------------------------------------------------------------
# NTFF profiling shim (only used when kernel(..., _trace=True); provides
# antenv.axon_hooks so run_bass_kernel_spmd can capture profiles under axon).
def _install_ntff_shim():
    import contextlib, ctypes, sys, types
    try:
        import antenv.axon_hooks  # noqa: F401
        return
    except ImportError:
        pass
    so = "/opt/axon/libaxon_pjrt.so"
    try:
        lib = ctypes.CDLL(so)
    except OSError:
        return
    if not hasattr(lib, "axon_start_nrt_profile"):
        return
    lib.axon_start_nrt_profile.argtypes = [
        ctypes.POINTER(ctypes.c_int64), ctypes.c_size_t]
    lib.axon_start_nrt_profile.restype = ctypes.c_int64
    lib.axon_stop_nrt_profile.argtypes = [ctypes.c_char_p]
    lib.axon_stop_nrt_profile.restype = ctypes.c_int64

    @contextlib.contextmanager
    def _hook(output_dir, device_ids):
        import jax
        jax.devices()
        if device_ids:
            ids = (ctypes.c_int64 * len(device_ids))(*device_ids)
            rc = lib.axon_start_nrt_profile(ids, len(device_ids))
        else:
            rc = lib.axon_start_nrt_profile(None, 0)
        if rc != 0:
            raise RuntimeError(f"axon_start_nrt_profile rc={rc}")
        try:
            yield
        finally:
            n = lib.axon_stop_nrt_profile(str(output_dir).encode())
            print(f"ntff: {n} profile file(s) in {output_dir}", file=sys.stderr)

    import antenv
    mod = types.ModuleType("antenv.axon_hooks")
    mod.get_axon_ntff_profile_hook = lambda: _hook
    mod.set_axon_ntff_profile_hook = lambda h: None
    sys.modules["antenv.axon_hooks"] = mod
    antenv.axon_hooks = mod


def kernel(x, mask, Wq, bq, Wk, bk, Wv, bv, Wo, bo, _trace=False):
    global LAST_EXEC_NS
    x = np.ascontiguousarray(np.asarray(x, dtype=np.float32))
    mask = np.asarray(mask)
    Wq = np.asarray(Wq, dtype=np.float32)
    Wk = np.asarray(Wk, dtype=np.float32)
    Wv = np.asarray(Wv, dtype=np.float32)
    Wo = np.asarray(Wo, dtype=np.float32)
    bq = np.asarray(bq, dtype=np.float32)
    bk = np.asarray(bk, dtype=np.float32)
    bv = np.asarray(bv, dtype=np.float32)
    bo = np.asarray(bo, dtype=np.float32)

    scale = np.float32(1.0 / math.sqrt(DK))
    apply_mask = not bool((mask != 0).all())
    qkv_bias = bool(bq.any() or bk.any() or bv.any())

    nc = _build(apply_mask, qkv_bias)

    if apply_mask:
        mbias = np.where(mask == 0, np.float32(-1e9), np.float32(0.0))
        # maskT[b][k, q] = mbias[b][q, k]
        maskT = np.ascontiguousarray(np.transpose(mbias, (0, 2, 1)))

    in_maps = []
    for b in range(B):
        xT_np = np.ascontiguousarray(x[b].T)  # [D, S]
        for g in range(HG):
            rows = slice(g * HD, (g + 1) * HD)
            wqkv_np = np.ascontiguousarray(np.concatenate(
                [Wq[rows].T * scale, Wk[rows].T, Wv[rows].T], axis=1))
            wo_np = np.ascontiguousarray(
                Wo[:, rows].T).astype(ml_dtypes.bfloat16)
            m = {"xT": xT_np, "wqkv": wqkv_np, "wo": wo_np}
            if apply_mask:
                m["maskT"] = maskT[b]
            if qkv_bias:
                m["qkb"] = np.ascontiguousarray(
                    np.stack([bq[rows] * scale, bk[rows]]))
                m["vb"] = np.ascontiguousarray(bv[rows])
            in_maps.append(m)

    if _trace:
        _install_ntff_shim()
    r = run_bass_kernel_spmd(nc, in_maps, list(range(NCORES)), trace=_trace)
    LAST_EXEC_NS = r.exec_time_ns

    y = np.empty((B, S, D), dtype=np.float32)
    for b in range(B):
        yT = r.results[2 * b]["yT"] + r.results[2 * b + 1]["yT"]
        y[b] = yT.T + bo[None, :]
    return y
